# revision 14
# baseline (speedup 1.0000x reference)
"""Trainium2 Bass kernel for nn_Division_Tree.

Reference semantics: a balanced binary "division tree" over X[0] of shape
[65536, 64].  128 leaves of 512 rows each get a dense 512x512 Gaussian-RBF
kernel (values, COO indices).  Every internal node appends a mean node
(mean over ALL nodes of its subtree, including previously appended means)
and emits new_v = RBF(subtree nodes, mean) twice plus a 1.0 diagonal entry.

Sharding: the 8 depth-4 subtrees (16 leaves each) map 1:1 onto the 8
NeuronCores.  Each core receives its shard pre-transposed (features major),
computes its 16 leaf kernels, its 15 local means/new_v rows, and its slice
of the depth-1..3 summary rows; only the per-core subtree sums (a [64]
vector each) are exchanged with a tiny AllGather.  COO indices are input
independent and built host-side; host assembly only places device-computed
values (no host math on X).

RBF trick used throughout: exp(-d2/64) = exp((x.y - |x|^2/2 - |y|^2/2)/32),
evaluated as two accumulated TensorE matmuls (Gram + rank-2 norm terms)
followed by a single fused ScalarE Exp(scale=1/32) straight out of PSUM.
"""

import numpy as np

# ---------------------------------------------------------------------------
# Static problem geometry (hardcoded; must match reference.py)
# ---------------------------------------------------------------------------
N = 65536
FMAP = 64
LEAF = 512            # rows per leaf (depth 8)
NLEAF = 128
NCORES = 8
LPC = NLEAF // NCORES         # 16 leaves per core
RPC = N // NCORES             # 8192 rows per core
CORE_NODES = RPC + LPC - 1  # 8207 nodes per core subtree (incl. its mean)
NNZ = 34473347
TOTAL_NODES = 65663


def _build_static():
    """Mirror of reference._dfs structure (no X dependence).

    Returns a dict of layout tables used for host-side assembly plus the
    exact COO indices array.
    """
    # ---- global recursion over the tree -----------------------------------
    leaf_val_off = []          # per leaf (in order): offset of its 512*512 block
    internal = []              # records for every internal node
    x_node_idx = np.empty(N, dtype=np.int64)   # global node index of each X row
    idx_parts = []             # pieces of the COO indices array

    def rec(x0, depth, n, node0, val0):
        """Subtree over X rows [x0, x0+n) at `depth`.

        node0/val0: global node-index / value-offset where this subtree's
        output starts.  Returns (node_count, nnz) of the subtree.
        """
        if n <= 30 or depth == 8:
            leaf_val_off.append(val0)
            x_node_idx[x0:x0 + n] = node0 + np.arange(n)
            r = np.arange(n)
            # ancestors' "+ left-node-count" shifts telescope to node0
            idx_parts.append(np.stack([np.repeat(r, n), np.tile(r, n)]) + node0)
            return n, n * n
        nl, zl = rec(x0, depth + 1, n // 2, node0, val0)
        nr, zr = rec(x0 + n // 2, depth + 1, n // 2, node0 + nl, val0 + zl)
        m = nl + nr
        ar = np.arange(m)
        full = np.full(m, m)
        idx_parts.append(np.concatenate(
            [np.stack([ar, full]), np.stack([full, ar]),
             np.array([[m], [m]])], 1) + node0)
        internal.append(dict(
            depth=depth, x0=x0, n=n, node0=node0, m=m,
            off1=val0 + zl + zr, off2=val0 + zl + zr + m,
            one_off=val0 + zl + zr + 2 * m,
            mean_node=node0 + m,
        ))
        return m + 1, zl + zr + 2 * m + 1

    total_nodes, nnz = rec(0, 1, N, 0, 0)
    assert total_nodes == TOTAL_NODES and nnz == NNZ, (total_nodes, nnz)

    # reference concatenates leaf/internal index blocks depth-first; the
    # recursion above appends idx_parts in exactly that order, but index
    # blocks of a PARENT must add the left-subtree offset to right-child
    # parts only (done above).  Base indices are subtree-local like the
    # reference (leaf blocks use local 0..n-1; parent adds offsets lazily
    # through the recursion).  The reference applies offsets the same way.
    indices = np.concatenate(idx_parts, axis=1).astype(np.int32)
    assert indices.shape == (2, NNZ)

    # ---- per-core tables ---------------------------------------------------
    # local means of core c in node-order appearance == sorted by mean_node
    local_nodes = [[] for _ in range(NCORES)]
    upper_nodes = []
    for rec_ in internal:
        if rec_["depth"] >= 4:
            c = rec_["x0"] // RPC
            local_nodes[c].append(rec_)
        else:
            upper_nodes.append(rec_)
    for c in range(NCORES):
        local_nodes[c].sort(key=lambda r: r["mean_node"])
        assert len(local_nodes[c]) == LPC - 1
    # start of each core's depth-4 subtree in global node order (upper mean
    # nodes are interleaved between core blocks, so this is NOT c*CORE_NODES)
    core_node0 = [local_nodes[c][-1]["node0"] for c in range(NCORES)]
    for c in range(NCORES):
        assert local_nodes[c][-1]["depth"] == 4
        assert local_nodes[c][-1]["mean_node"] == core_node0[c] + CORE_NODES - 1
    # mean id -> (owning first core, uvals/mm row index)
    mean_row = {}
    for c in range(NCORES):
        for r, rec_ in enumerate(local_nodes[c]):
            mean_row[rec_["mean_node"]] = (c, r)
    for rec_ in upper_nodes:
        c0 = rec_["x0"] // RPC
        mean_row[rec_["mean_node"]] = (c0, 18 - rec_["depth"])

    # node-order position of each x row / local mean inside its core block
    xpos = [x_node_idx[c * RPC:(c + 1) * RPC] - core_node0[c]
            for c in range(NCORES)]
    mpos = [np.array([r_["mean_node"] - core_node0[c]
                      for r_ in local_nodes[c]], dtype=np.int64)
            for c in range(NCORES)]

    # ---- C16: local mean/sum coefficients over the 16 leaf sums -----------
    means_coef = []

    def lrec(lo, hi, depth):
        if depth == 8:
            e = np.zeros(LPC)
            e[lo] = 1.0
            return e, 512
        mid = (lo + hi) // 2
        tl, cl = lrec(lo, mid, depth + 1)
        tr, cr = lrec(mid, hi, depth + 1)
        mean = (tl + tr) / (cl + cr)
        means_coef.append(mean)
        return tl + tr + mean, cl + cr + 1

    t4, c4 = lrec(0, LPC, 4)
    assert c4 == CORE_NODES and len(means_coef) == 15
    c16 = np.stack(means_coef + [t4], axis=1).astype(np.float32)  # [16,16]

    # ---- CU: upper mean coefficients over the 8 core sums -----------------
    upper_coef = {}   # (depth, first_core) -> coef [8]

    def urec(lo, hi, depth):
        if depth == 4:
            e = np.zeros(NCORES)
            e[lo] = 1.0
            return e, CORE_NODES
        mid = (lo + hi) // 2
        tl, cl = urec(lo, mid, depth + 1)
        tr, cr = urec(mid, hi, depth + 1)
        mean = (tl + tr) / (cl + cr)
        upper_coef[(depth, lo)] = mean
        return tl + tr + mean, cl + cr + 1

    _, c1 = urec(0, NCORES, 1)
    assert c1 == TOTAL_NODES
    cu = np.zeros((NCORES, 8, 3), dtype=np.float32)
    for c in range(NCORES):
        cu[c, :, 0] = upper_coef[(3, (c // 2) * 2)]
        cu[c, :, 1] = upper_coef[(2, (c // 4) * 4)]
        cu[c, :, 2] = upper_coef[(1, 0)]

    return dict(
        indices=indices,
        leaf_val_off=np.array(leaf_val_off, dtype=np.int64),
        local_nodes=local_nodes,
        upper_nodes=upper_nodes,
        mean_row=mean_row,
        xpos=xpos,
        mpos=mpos,
        core_node0=core_node0,
        x_node_idx=x_node_idx,
        c16=c16,
        cu=cu,
    )


_STATIC = None


def _static():
    global _STATIC
    if _STATIC is None:
        _STATIC = _build_static()
    return _STATIC


# ---------------------------------------------------------------------------
# Device kernel
# ---------------------------------------------------------------------------
_NC = None
_LAST_RESULTS = None  # test.py reads .exec_time_ns when BASS_TRACE is set


def _build_nc():
    from contextlib import ExitStack
    import concourse.tile as tile
    from concourse import bacc, mybir

    F32 = mybir.dt.float32
    F32R = mybir.dt.float32r
    EXP = mybir.ActivationFunctionType.Exp

    nc = bacc.Bacc("TRN2", target_bir_lowering=False, debug=False,
                   num_devices=NCORES)

    xt_d = nc.dram_tensor("xt", [FMAP, RPC], F32R, kind="ExternalInput").ap()
    c16_d = nc.dram_tensor("c16", [LPC, 16], F32, kind="ExternalInput").ap()
    cu_d = nc.dram_tensor("cu", [8, 3], F32, kind="ExternalInput").ap()
    eye_d = nc.dram_tensor("eye64", [64, 64], F32, kind="ExternalInput").ap()
    w2_d = nc.dram_tensor("w2", [65, 2], F32R, kind="ExternalInput").ap()
    w2l_d = nc.dram_tensor("w2l", [65, 2], F32R, kind="ExternalInput").ap()
    v2_d = nc.dram_tensor("v2", [65, 2], F32, kind="ExternalInput").ap()

    lv_d = nc.dram_tensor("leafvals", [LPC, LEAF, LEAF], F32,
                          kind="ExternalOutput").ap()
    uv_d = nc.dram_tensor("uvals", [18, RPC], F32,
                          kind="ExternalOutput").ap()
    mm_d = nc.dram_tensor("mmout", [18, 18], F32, kind="ExternalOutput").ap()
    me_d = nc.dram_tensor("meansout", [FMAP, 18], F32,
                          kind="ExternalOutput").ap()

    with tile.TileContext(nc) as tc, ExitStack() as ctx:
        sb = ctx.enter_context(tc.tile_pool(name="sb", bufs=1))
        kpool = ctx.enter_context(tc.tile_pool(name="kbig", bufs=3))
        kp = ctx.enter_context(tc.tile_pool(name="kp", bufs=3, space="PSUM"))
        up = ctx.enter_context(tc.tile_pool(name="up", bufs=3, space="PSUM"))
        sp = ctx.enter_context(tc.tile_pool(name="sp", bufs=2, space="PSUM"))
        dram = ctx.enter_context(tc.tile_pool(name="dram", bufs=1,
                                              space="DRAM"))

        # ---- persistent SBUF tiles ----------------------------------------
        xl = sb.tile([66, RPC], F32R, tag="xl")     # [x ; -0.5 ; a]
        xr = sb.tile([66, RPC], F32R, tag="xr")     # [x ; a ; -0.5]
        xsq = sb.tile([65, RPC], F32R, tag="xsq")   # [x*x ; 1]
        ls = sb.tile([64, 16], F32, tag="ls")
        lst = sb.tile([16, 64], F32, tag="lst")
        eye = sb.tile([64, 64], F32, tag="eye")
        c16s = sb.tile([16, 16], F32, tag="c16s")
        cus = sb.tile([8, 3], F32, tag="cus")
        w2s = sb.tile([65, 2], F32R, tag="w2s")
        w2ls = sb.tile([65, 2], F32R, tag="w2ls")
        v2s = sb.tile([65, 2], F32, tag="v2s")
        at4 = sb.tile([8, 64], F32, tag="at4")
        augl = sb.tile([66, 18], F32R, tag="augl")  # [means ; -0.5 ; |m|^2]
        msq65 = sb.tile([65, 18], F32, tag="msq65")
        mn2r = sb.tile([1, 18], F32, tag="mn2r")
        ones_r = sb.tile([1, 32], F32, tag="ones_r")
        ones64 = sb.tile([64, 1], F32, tag="ones64")
        t4sb = sb.tile([64, 1], F32, tag="t4sb")
        mmsb = sb.tile([18, 18], F32, tag="mmsb")
        meansf = sb.tile([64, 18], F32, tag="meansf")
        ubig = sb.tile([18, RPC], F32, tag="ubig")

        nc.sync.dma_start(xl[0:64, :], xt_d[:])
        nc.sync.dma_start(eye[:], eye_d[:])
        nc.sync.dma_start(c16s[:], c16_d[:])
        nc.sync.dma_start(cus[:], cu_d[:])
        nc.sync.dma_start(w2s[:], w2_d[:])
        nc.sync.dma_start(w2ls[:], w2l_d[:])
        nc.sync.dma_start(v2s[:], v2_d[:])
        nc.gpsimd.memset(xsq[64:65, :].bitcast(mybir.dt.uint32), 0x3F800000)
        nc.gpsimd.memset(ones_r[:], 1.0)
        nc.gpsimd.memset(ones64[:], 1.0)

        nc.vector.tensor_copy(xr[0:64, :], xl[0:64, :])
        nc.vector.tensor_mul(xsq[0:64, :], xl[0:64, :], xl[0:64, :])
        nc.vector.reduce_sum(ls[:],
                             xl[0:64, :].bitcast(F32)
                             .rearrange("p (l n) -> p l n", l=16),
                             axis=mybir.AxisListType.X)

        # ---- local means + core total sum ---------------------------------
        lstp = sp.tile([16, 64], F32, tag="sp")
        nc.tensor.transpose(lstp[:], ls[:], eye[:])
        nc.vector.tensor_copy(lst[:], lstp[:])
        meansp = sp.tile([64, 16], F32, tag="sp")
        nc.tensor.matmul(meansp[:], lst[:], c16s[:], start=True, stop=True)
        nc.vector.tensor_copy(augl[0:64, 0:15], meansp[:, 0:15])
        nc.vector.tensor_copy(meansf[:, 0:15], meansp[:, 0:15])
        nc.vector.tensor_copy(t4sb[:], meansp[:, 15:16])

        # ---- AllGather of core subtree sums -------------------------------
        cc_in = dram.tile([64, 1], F32)
        cc_out = dram.tile([NCORES * 64, 1], F32)
        nc.sync.dma_start(cc_in[:], t4sb[:])
        nc.gpsimd.collective_compute(
            "AllGather", mybir.AluOpType.bypass,
            replica_groups=[list(range(NCORES))],
            ins=[cc_in.opt()], outs=[cc_out.opt()])
        nc.sync.dma_start(at4[:],
                          cc_out[:].rearrange("(c f) o -> c (f o)", c=NCORES))
        mupp = sp.tile([64, 3], F32, tag="sp")
        nc.tensor.matmul(mupp[:], at4[:], cus[:], start=True, stop=True)
        nc.vector.tensor_copy(augl[0:64, 15:18], mupp[:])
        nc.vector.tensor_copy(meansf[:, 15:18], mupp[:])

        # ---- mean norms ----------------------------------------------------
        nc.vector.tensor_mul(msq65[0:64, 0:18], augl[0:64, 0:18],
                             augl[0:64, 0:18])
        nc.gpsimd.memset(msq65[64:65, 0:18], 1.0)
        mnp = sp.tile([1, 18], F32, tag="sp")
        nc.tensor.matmul(mnp[:], ones64[:], msq65[0:64, 0:18],
                         start=True, stop=True)
        nc.vector.tensor_scalar_mul(mn2r[:], mnp[:], -0.5)
        pt2m = sp.tile([2, 18], F32, tag="sp")
        nc.tensor.matmul(pt2m[:], v2s[:], msq65[:, 0:18], start=True, stop=True)
        nc.vector.tensor_copy(augl[64:66, 0:18], pt2m[:])

        # ---- leaf RBF kernels ---------------------------------------------
        for l in range(LPC):
            lo = l * LEAF
            pt2r = sp.tile([2, LEAF], F32, tag="sp")
            nc.tensor.matmul(pt2r[:], w2s[:],
                             xsq[:, lo:lo + LEAF],
                             start=True, stop=True)
            nc.vector.tensor_copy(xr[64:66, lo:lo + LEAF], pt2r[:])
            pt2l = sp.tile([2, LEAF], F32, tag="sp")
            nc.tensor.matmul(pt2l[:], w2ls[:],
                             xsq[:, lo:lo + LEAF],
                             start=True, stop=True)
            nc.vector.tensor_copy(xl[64:66, lo:lo + LEAF], pt2l[:])

            kbig = kpool.tile([128, 4 * LEAF], F32, tag="kbig")
            for c in range(4):
                kps = kp.tile([128, LEAF], F32, tag="kp")
                col0 = lo + c * 128
                nc.tensor.matmul(kps[:], xl[:, col0:col0 + 128],
                                 xr[:, lo:lo + LEAF],
                                 start=True, stop=True)
                nc.scalar.activation(kbig[:, c * LEAF:(c + 1) * LEAF],
                                     kps[:], EXP, scale=1.0 / 32.0)
            nc.sync.dma_start(
                lv_d[l].rearrange("(cc p) n -> p cc n", p=128),
                kbig[:].rearrange("p (cc n) -> p cc n", cc=4))

        # ---- per-row summary values vs the 18 ancestor means --------------
        for ch in range(16):
            lo = ch * LEAF
            ups = up.tile([18, LEAF], F32, tag="up")
            nc.tensor.matmul(ups[:], augl[:, 0:18],
                             xr[:, lo:lo + LEAF],
                             start=True, stop=True)
            nc.scalar.activation(ubig[:, lo:lo + LEAF], ups[:], EXP,
                                 scale=1.0 / 32.0)
        nc.sync.dma_start(uv_d[:], ubig[:])

        # ---- mean-vs-mean summary values ----------------------------------
        mmp = sp.tile([18, 18], F32, tag="sp")
        nc.tensor.matmul(mmp[:], augl[0:64, 0:18], augl[0:64, 0:18],
                         start=True, stop=False)
        nc.tensor.matmul(mmp[:], mn2r[:], ones_r[0:1, 0:18],
                         start=False, stop=False)
        nc.tensor.matmul(mmp[:], ones_r[0:1, 0:18], mn2r[:],
                         start=False, stop=True)
        nc.scalar.activation(mmsb[:], mmp[:], EXP, scale=1.0 / 32.0)
        nc.sync.dma_start(mm_d[:], mmsb[:])
        nc.sync.dma_start(me_d[:], meansf[:])

    nc.compile()
    return nc


def _get_nc():
    global _NC
    if _NC is None:
        _NC = _build_nc()
    return _NC


# ---------------------------------------------------------------------------
# Host entry point
# ---------------------------------------------------------------------------
def kernel(X):
    global _LAST_RESULTS
    from concourse.bass_utils import run_bass_kernel_spmd

    st = _static()
    X = np.asarray(X)
    assert X.shape == (1, N, FMAP), X.shape
    x2 = X[0].astype(np.float32, copy=False)

    eye = np.eye(64, dtype=np.float32)
    w2 = np.zeros((65, 2), dtype=np.float32)   # xr rows 64-65 = [a ; -0.5]
    w2[0:64, 0] = 1.0
    w2[64, 1] = -0.5
    w2l = np.zeros((65, 2), dtype=np.float32)  # xl rows 64-65 = [-0.5 ; a]
    w2l[64, 0] = -0.5
    w2l[0:64, 1] = 1.0
    v2 = np.zeros((65, 2), dtype=np.float32)   # augl rows 64-65 = [-0.5 ; |m|^2]
    v2[64, 0] = -0.5
    v2[0:64, 1] = 1.0
    in_maps = []
    for c in range(NCORES):
        xt = np.ascontiguousarray(x2[c * RPC:(c + 1) * RPC].T)
        in_maps.append({"xt": xt, "c16": st["c16"], "cu": st["cu"][c],
                        "eye64": eye, "w2": w2, "w2l": w2l, "v2": v2})

    nc = _get_nc()
    res = run_bass_kernel_spmd(nc, in_maps, core_ids=list(range(NCORES)))
    _LAST_RESULTS = res
    outs = res.results

    # ---- assemble values ---------------------------------------------------
    values = np.empty(NNZ, dtype=np.float32)
    nv = []   # per-core [18, CORE_NODES] node-order summary rows
    for c in range(NCORES):
        lv = outs[c]["leafvals"]           # [16, 512, 512]
        base = c * LPC
        for l in range(LPC):
            off = st["leaf_val_off"][base + l]
            values[off:off + LEAF * LEAF] = lv[l].reshape(-1)
        nvc = np.empty((18, CORE_NODES), dtype=np.float32)
        nvc[:, st["xpos"][c]] = outs[c]["uvals"]
        nvc[:, st["mpos"][c]] = outs[c]["mmout"][:, 0:15]
        nv.append(nvc)

    core_node0 = st["core_node0"]
    for c in range(NCORES):
        for r, rec_ in enumerate(st["local_nodes"][c]):
            s = rec_["node0"] - core_node0[c]
            m = rec_["m"]
            seg = nv[c][r, s:s + m]
            values[rec_["off1"]:rec_["off1"] + m] = seg
            values[rec_["off2"]:rec_["off2"] + m] = seg
            values[rec_["one_off"]] = 1.0

    mean_row = st["mean_row"]
    for rec_ in st["upper_nodes"]:
        d = rec_["depth"]
        row = 18 - d
        c_lo = rec_["x0"] // RPC
        c_hi = (rec_["x0"] + rec_["n"]) // RPC
        for off in (rec_["off1"], rec_["off2"]):
            for c in range(c_lo, c_hi):
                dst = off + (core_node0[c] - rec_["node0"])
                values[dst:dst + CORE_NODES] = nv[c][row]
            # mean nodes of strictly deeper internal nodes within span
            for sub in st["upper_nodes"]:
                if sub["depth"] > d and rec_["x0"] <= sub["x0"] \
                        and sub["x0"] + sub["n"] <= rec_["x0"] + rec_["n"]:
                    sc, scol = mean_row[sub["mean_node"]]
                    values[off + (sub["mean_node"] - rec_["node0"])] = \
                        outs[sc]["mmout"][row, scol]
        values[rec_["one_off"]] = 1.0

    # ---- assemble all_nodes ------------------------------------------------
    nodes = np.empty((TOTAL_NODES, FMAP), dtype=np.float32)
    nodes[st["x_node_idx"]] = x2
    for c in range(NCORES):
        me = outs[c]["meansout"]           # [64, 18]
        for r, rec_ in enumerate(st["local_nodes"][c]):
            nodes[rec_["mean_node"]] = me[:, r]
    for rec_ in st["upper_nodes"]:
        sc, scol = mean_row[rec_["mean_node"]]
        nodes[rec_["mean_node"]] = outs[sc]["meansout"][:, scol]

    return nodes[None], st["indices"], values


# revision 16
# speedup vs baseline: 1.2181x; 1.2181x over previous
"""Trainium2 Bass kernel for nn_Division_Tree.

Reference semantics: a balanced binary "division tree" over X[0] of shape
[65536, 64].  128 leaves of 512 rows each get a dense 512x512 Gaussian-RBF
kernel (values, COO indices).  Every internal node appends a mean node
(mean over ALL nodes of its subtree, including previously appended means)
and emits new_v = RBF(subtree nodes, mean) twice plus a 1.0 diagonal entry.

Sharding: the 8 depth-4 subtrees (16 leaves each) map 1:1 onto the 8
NeuronCores.  Each core receives its shard pre-transposed (features major),
computes its 16 leaf kernels, its 15 local means/new_v rows, and its slice
of the depth-1..3 summary rows; only the per-core subtree sums (a [64]
vector each) are exchanged with a tiny AllGather.  COO indices are input
independent and built host-side; host assembly only places device-computed
values (no host math on X).

RBF trick used throughout: exp(-d2/64) = exp((x.y - |x|^2/2 - |y|^2/2)/32),
evaluated as two accumulated TensorE matmuls (Gram + rank-2 norm terms)
followed by a single fused ScalarE Exp(scale=1/32) straight out of PSUM.
"""

import numpy as np

# ---------------------------------------------------------------------------
# Static problem geometry (hardcoded; must match reference.py)
# ---------------------------------------------------------------------------
N = 65536
FMAP = 64
LEAF = 512            # rows per leaf (depth 8)
NLEAF = 128
NCORES = 8
LPC = NLEAF // NCORES         # 16 leaves per core
RPC = N // NCORES             # 8192 rows per core
CORE_NODES = RPC + LPC - 1  # 8207 nodes per core subtree (incl. its mean)
NNZ = 34473347
TOTAL_NODES = 65663


def _build_static():
    """Mirror of reference._dfs structure (no X dependence).

    Returns a dict of layout tables used for host-side assembly plus the
    exact COO indices array.
    """
    # ---- global recursion over the tree -----------------------------------
    leaf_val_off = []          # per leaf (in order): offset of its 512*512 block
    internal = []              # records for every internal node
    x_node_idx = np.empty(N, dtype=np.int64)   # global node index of each X row
    idx_parts = []             # pieces of the COO indices array

    def rec(x0, depth, n, node0, val0):
        """Subtree over X rows [x0, x0+n) at `depth`.

        node0/val0: global node-index / value-offset where this subtree's
        output starts.  Returns (node_count, nnz) of the subtree.
        """
        if n <= 30 or depth == 8:
            leaf_val_off.append(val0)
            x_node_idx[x0:x0 + n] = node0 + np.arange(n)
            r = np.arange(n)
            # ancestors' "+ left-node-count" shifts telescope to node0
            idx_parts.append(np.stack([np.repeat(r, n), np.tile(r, n)]) + node0)
            return n, n * n
        nl, zl = rec(x0, depth + 1, n // 2, node0, val0)
        nr, zr = rec(x0 + n // 2, depth + 1, n // 2, node0 + nl, val0 + zl)
        m = nl + nr
        ar = np.arange(m)
        full = np.full(m, m)
        idx_parts.append(np.concatenate(
            [np.stack([ar, full]), np.stack([full, ar]),
             np.array([[m], [m]])], 1) + node0)
        internal.append(dict(
            depth=depth, x0=x0, n=n, node0=node0, m=m,
            off1=val0 + zl + zr, off2=val0 + zl + zr + m,
            one_off=val0 + zl + zr + 2 * m,
            mean_node=node0 + m,
        ))
        return m + 1, zl + zr + 2 * m + 1

    total_nodes, nnz = rec(0, 1, N, 0, 0)
    assert total_nodes == TOTAL_NODES and nnz == NNZ, (total_nodes, nnz)

    # reference concatenates leaf/internal index blocks depth-first; the
    # recursion above appends idx_parts in exactly that order, but index
    # blocks of a PARENT must add the left-subtree offset to right-child
    # parts only (done above).  Base indices are subtree-local like the
    # reference (leaf blocks use local 0..n-1; parent adds offsets lazily
    # through the recursion).  The reference applies offsets the same way.
    indices = np.concatenate(idx_parts, axis=1).astype(np.int32)
    assert indices.shape == (2, NNZ)

    # ---- per-core tables ---------------------------------------------------
    # local means of core c in node-order appearance == sorted by mean_node
    local_nodes = [[] for _ in range(NCORES)]
    upper_nodes = []
    for rec_ in internal:
        if rec_["depth"] >= 4:
            c = rec_["x0"] // RPC
            local_nodes[c].append(rec_)
        else:
            upper_nodes.append(rec_)
    for c in range(NCORES):
        local_nodes[c].sort(key=lambda r: r["mean_node"])
        assert len(local_nodes[c]) == LPC - 1
    # start of each core's depth-4 subtree in global node order (upper mean
    # nodes are interleaved between core blocks, so this is NOT c*CORE_NODES)
    core_node0 = [local_nodes[c][-1]["node0"] for c in range(NCORES)]
    for c in range(NCORES):
        assert local_nodes[c][-1]["depth"] == 4
        assert local_nodes[c][-1]["mean_node"] == core_node0[c] + CORE_NODES - 1
    # mean id -> (owning first core, uvals/mm row index)
    mean_row = {}
    for c in range(NCORES):
        for r, rec_ in enumerate(local_nodes[c]):
            mean_row[rec_["mean_node"]] = (c, r)
    for rec_ in upper_nodes:
        c0 = rec_["x0"] // RPC
        mean_row[rec_["mean_node"]] = (c0, 18 - rec_["depth"])

    # node-order position of each x row / local mean inside its core block
    xpos = [x_node_idx[c * RPC:(c + 1) * RPC] - core_node0[c]
            for c in range(NCORES)]
    mpos = [np.array([r_["mean_node"] - core_node0[c]
                      for r_ in local_nodes[c]], dtype=np.int64)
            for c in range(NCORES)]

    # ---- C16: local mean/sum coefficients over the 16 leaf sums -----------
    means_coef = []

    def lrec(lo, hi, depth):
        if depth == 8:
            e = np.zeros(LPC)
            e[lo] = 1.0
            return e, 512
        mid = (lo + hi) // 2
        tl, cl = lrec(lo, mid, depth + 1)
        tr, cr = lrec(mid, hi, depth + 1)
        mean = (tl + tr) / (cl + cr)
        means_coef.append(mean)
        return tl + tr + mean, cl + cr + 1

    t4, c4 = lrec(0, LPC, 4)
    assert c4 == CORE_NODES and len(means_coef) == 15
    c16 = np.stack(means_coef + [t4], axis=1).astype(np.float32)  # [16,16]

    # ---- CU: upper mean coefficients over the 8 core sums -----------------
    upper_coef = {}   # (depth, first_core) -> coef [8]

    def urec(lo, hi, depth):
        if depth == 4:
            e = np.zeros(NCORES)
            e[lo] = 1.0
            return e, CORE_NODES
        mid = (lo + hi) // 2
        tl, cl = urec(lo, mid, depth + 1)
        tr, cr = urec(mid, hi, depth + 1)
        mean = (tl + tr) / (cl + cr)
        upper_coef[(depth, lo)] = mean
        return tl + tr + mean, cl + cr + 1

    _, c1 = urec(0, NCORES, 1)
    assert c1 == TOTAL_NODES
    cu = np.zeros((NCORES, 8, 3), dtype=np.float32)
    for c in range(NCORES):
        cu[c, :, 0] = upper_coef[(3, (c // 2) * 2)]
        cu[c, :, 1] = upper_coef[(2, (c // 4) * 4)]
        cu[c, :, 2] = upper_coef[(1, 0)]

    return dict(
        indices=indices,
        leaf_val_off=np.array(leaf_val_off, dtype=np.int64),
        local_nodes=local_nodes,
        upper_nodes=upper_nodes,
        mean_row=mean_row,
        xpos=xpos,
        mpos=mpos,
        core_node0=core_node0,
        x_node_idx=x_node_idx,
        c16=c16,
        cu=cu,
    )


_STATIC = None


def _static():
    global _STATIC
    if _STATIC is None:
        _STATIC = _build_static()
    return _STATIC


# ---------------------------------------------------------------------------
# Device kernel
# ---------------------------------------------------------------------------
_NC = None
_LAST_RESULTS = None  # test.py reads .exec_time_ns when BASS_TRACE is set


def _build_nc():
    from contextlib import ExitStack
    import concourse.tile as tile
    from concourse import bacc, mybir

    F32 = mybir.dt.float32
    F32R = mybir.dt.float32r
    EXP = mybir.ActivationFunctionType.Exp

    nc = bacc.Bacc("TRN2", target_bir_lowering=False, debug=False,
                   num_devices=NCORES)

    xt_d = nc.dram_tensor("xt", [FMAP, RPC], F32R, kind="ExternalInput").ap()
    c16_d = nc.dram_tensor("c16", [LPC, 16], F32, kind="ExternalInput").ap()
    cu_d = nc.dram_tensor("cu", [8, 3], F32, kind="ExternalInput").ap()
    eye_d = nc.dram_tensor("eye64", [64, 64], F32, kind="ExternalInput").ap()
    w2_d = nc.dram_tensor("w2", [65, 2], F32R, kind="ExternalInput").ap()
    w2l_d = nc.dram_tensor("w2l", [65, 2], F32R, kind="ExternalInput").ap()
    v2_d = nc.dram_tensor("v2", [65, 2], F32, kind="ExternalInput").ap()

    lv_d = nc.dram_tensor("leafvals", [LPC, LEAF, LEAF], F32,
                          kind="ExternalOutput").ap()
    uv_d = nc.dram_tensor("uvals", [18, RPC], F32,
                          kind="ExternalOutput").ap()
    mm_d = nc.dram_tensor("mmout", [18, 18], F32, kind="ExternalOutput").ap()
    me_d = nc.dram_tensor("meansout", [FMAP, 18], F32,
                          kind="ExternalOutput").ap()

    with tile.TileContext(nc) as tc, ExitStack() as ctx:
        sb = ctx.enter_context(tc.tile_pool(name="sb", bufs=1))
        kpool = ctx.enter_context(tc.tile_pool(name="kbig", bufs=3))
        kp = ctx.enter_context(tc.tile_pool(name="kp", bufs=3, space="PSUM"))
        up = ctx.enter_context(tc.tile_pool(name="up", bufs=3, space="PSUM"))
        sp = ctx.enter_context(tc.tile_pool(name="sp", bufs=2, space="PSUM"))
        dram = ctx.enter_context(tc.tile_pool(name="dram", bufs=1,
                                              space="DRAM"))

        # ---- persistent SBUF tiles ----------------------------------------
        xl = sb.tile([66, RPC], F32R, tag="xl")     # [x ; -0.5 ; a]
        xr = sb.tile([66, RPC], F32R, tag="xr")     # [x ; a ; -0.5]
        xsq = sb.tile([65, RPC], F32R, tag="xsq")   # [x*x ; 1]
        ls = sb.tile([64, 16], F32, tag="ls")
        lst = sb.tile([16, 64], F32, tag="lst")
        eye = sb.tile([64, 64], F32, tag="eye")
        c16s = sb.tile([16, 16], F32, tag="c16s")
        cus = sb.tile([8, 3], F32, tag="cus")
        w2s = sb.tile([65, 2], F32R, tag="w2s")
        w2ls = sb.tile([65, 2], F32R, tag="w2ls")
        v2s = sb.tile([65, 2], F32, tag="v2s")
        at4 = sb.tile([8, 64], F32, tag="at4")
        augl = sb.tile([66, 18], F32R, tag="augl")  # [means ; -0.5 ; |m|^2]
        msq65 = sb.tile([65, 18], F32, tag="msq65")
        mn2r = sb.tile([1, 18], F32, tag="mn2r")
        ones_r = sb.tile([1, 32], F32, tag="ones_r")
        ones64 = sb.tile([64, 1], F32, tag="ones64")
        t4sb = sb.tile([64, 1], F32, tag="t4sb")
        mmsb = sb.tile([18, 18], F32, tag="mmsb")
        meansf = sb.tile([64, 18], F32, tag="meansf")
        ubig = sb.tile([18, RPC], F32, tag="ubig")

        nc.sync.dma_start(xl[0:64, :], xt_d[:])
        nc.sync.dma_start(eye[:], eye_d[:])
        nc.sync.dma_start(c16s[:], c16_d[:])
        nc.sync.dma_start(cus[:], cu_d[:])
        nc.sync.dma_start(w2s[:], w2_d[:])
        nc.sync.dma_start(w2ls[:], w2l_d[:])
        nc.sync.dma_start(v2s[:], v2_d[:])
        nc.gpsimd.memset(xsq[64:65, :].bitcast(mybir.dt.uint32), 0x3F800000)
        nc.gpsimd.memset(ones_r[:], 1.0)
        nc.gpsimd.memset(ones64[:], 1.0)

        # LS reduce goes FIRST on the DVE queue: the collective depends on it
        # and must launch while the leaf stream crunches.
        nc.vector.reduce_sum(ls[:],
                             xl[0:64, :].bitcast(F32)
                             .rearrange("p (l n) -> p l n", l=16),
                             axis=mybir.AxisListType.X)

        # ---- local means + core total sum ---------------------------------
        lstp = sp.tile([16, 64], F32, tag="sp")
        nc.tensor.transpose(lstp[:], ls[:], eye[:])
        nc.vector.tensor_copy(lst[:], lstp[:])
        meansp = sp.tile([64, 16], F32, tag="sp")
        nc.tensor.matmul(meansp[:], lst[:], c16s[:], start=True, stop=True)
        nc.vector.tensor_copy(augl[0:64, 0:15], meansp[:, 0:15])
        nc.vector.tensor_copy(meansf[:, 0:15], meansp[:, 0:15])
        nc.vector.tensor_copy(t4sb[:], meansp[:, 15:16])

        # ---- AllGather of core subtree sums (completes under leaf stream) --
        cc_in = dram.tile([64, 1], F32)
        cc_out = dram.tile([NCORES * 64, 1], F32)
        nc.sync.dma_start(cc_in[:], t4sb[:])
        nc.gpsimd.collective_compute(
            "AllGather", mybir.AluOpType.bypass,
            replica_groups=[list(range(NCORES))],
            ins=[cc_in.opt()], outs=[cc_out.opt()])
        nc.sync.dma_start(at4[:],
                          cc_out[:].rearrange("(c f) o -> c (f o)", c=NCORES))

        nc.vector.tensor_mul(xsq[0:64, :], xl[0:64, :], xl[0:64, :])
        nc.vector.tensor_copy(xr[0:64, :], xl[0:64, :])

        # ---- leaf RBF kernels ---------------------------------------------
        for l in range(LPC):
            lo = l * LEAF
            pt2r = sp.tile([2, LEAF], F32, tag="sp")
            nc.tensor.matmul(pt2r[:], w2s[:],
                             xsq[:, lo:lo + LEAF],
                             start=True, stop=True)
            nc.vector.tensor_copy(xr[64:66, lo:lo + LEAF], pt2r[:])
            pt2l = sp.tile([2, LEAF], F32, tag="sp")
            nc.tensor.matmul(pt2l[:], w2ls[:],
                             xsq[:, lo:lo + LEAF],
                             start=True, stop=True)
            nc.vector.tensor_copy(xl[64:66, lo:lo + LEAF], pt2l[:])

            kbig = kpool.tile([128, 4 * LEAF], F32, tag="kbig")
            for c in range(4):
                kps = kp.tile([128, LEAF], F32, tag="kp")
                col0 = lo + c * 128
                nc.tensor.matmul(kps[:], xl[:, col0:col0 + 128],
                                 xr[:, lo:lo + LEAF],
                                 start=True, stop=True)
                nc.scalar.activation(kbig[:, c * LEAF:(c + 1) * LEAF],
                                     kps[:], EXP, scale=1.0 / 32.0)
            nc.sync.dma_start(
                lv_d[l].rearrange("(cc p) n -> p cc n", p=128),
                kbig[:].rearrange("p (cc n) -> p cc n", cc=4))

        # ---- upper means + mean norms (collective landed long ago) --------
        mupp = sp.tile([64, 3], F32, tag="sp")
        nc.tensor.matmul(mupp[:], at4[:], cus[:], start=True, stop=True)
        nc.vector.tensor_copy(augl[0:64, 15:18], mupp[:])
        nc.vector.tensor_copy(meansf[:, 15:18], mupp[:])

        nc.vector.tensor_mul(msq65[0:64, 0:18], augl[0:64, 0:18],
                             augl[0:64, 0:18])
        nc.gpsimd.memset(msq65[64:65, 0:18], 1.0)
        mnp = sp.tile([1, 18], F32, tag="sp")
        nc.tensor.matmul(mnp[:], ones64[:], msq65[0:64, 0:18],
                         start=True, stop=True)
        nc.vector.tensor_scalar_mul(mn2r[:], mnp[:], -0.5)
        pt2m = sp.tile([2, 18], F32, tag="sp")
        nc.tensor.matmul(pt2m[:], v2s[:], msq65[:, 0:18], start=True, stop=True)
        nc.vector.tensor_copy(augl[64:66, 0:18], pt2m[:])

        # ---- per-row summary values vs the 18 ancestor means --------------
        for ch in range(16):
            lo = ch * LEAF
            ups = up.tile([18, LEAF], F32, tag="up")
            nc.tensor.matmul(ups[:], augl[:, 0:18],
                             xr[:, lo:lo + LEAF],
                             start=True, stop=True)
            nc.scalar.activation(ubig[:, lo:lo + LEAF], ups[:], EXP,
                                 scale=1.0 / 32.0)
        nc.sync.dma_start(uv_d[:], ubig[:])

        # ---- mean-vs-mean summary values ----------------------------------
        mmp = sp.tile([18, 18], F32, tag="sp")
        nc.tensor.matmul(mmp[:], augl[0:64, 0:18], augl[0:64, 0:18],
                         start=True, stop=False)
        nc.tensor.matmul(mmp[:], mn2r[:], ones_r[0:1, 0:18],
                         start=False, stop=False)
        nc.tensor.matmul(mmp[:], ones_r[0:1, 0:18], mn2r[:],
                         start=False, stop=True)
        nc.scalar.activation(mmsb[:], mmp[:], EXP, scale=1.0 / 32.0)
        nc.sync.dma_start(mm_d[:], mmsb[:])
        nc.sync.dma_start(me_d[:], meansf[:])

    nc.compile()
    return nc


def _get_nc():
    global _NC
    if _NC is None:
        _NC = _build_nc()
    return _NC


# ---------------------------------------------------------------------------
# Host entry point
# ---------------------------------------------------------------------------
def kernel(X):
    global _LAST_RESULTS
    from concourse.bass_utils import run_bass_kernel_spmd

    st = _static()
    X = np.asarray(X)
    assert X.shape == (1, N, FMAP), X.shape
    x2 = X[0].astype(np.float32, copy=False)

    eye = np.eye(64, dtype=np.float32)
    w2 = np.zeros((65, 2), dtype=np.float32)   # xr rows 64-65 = [a ; -0.5]
    w2[0:64, 0] = 1.0
    w2[64, 1] = -0.5
    w2l = np.zeros((65, 2), dtype=np.float32)  # xl rows 64-65 = [-0.5 ; a]
    w2l[64, 0] = -0.5
    w2l[0:64, 1] = 1.0
    v2 = np.zeros((65, 2), dtype=np.float32)   # augl rows 64-65 = [-0.5 ; |m|^2]
    v2[64, 0] = -0.5
    v2[0:64, 1] = 1.0
    in_maps = []
    for c in range(NCORES):
        xt = np.ascontiguousarray(x2[c * RPC:(c + 1) * RPC].T)
        in_maps.append({"xt": xt, "c16": st["c16"], "cu": st["cu"][c],
                        "eye64": eye, "w2": w2, "w2l": w2l, "v2": v2})

    nc = _get_nc()
    res = run_bass_kernel_spmd(nc, in_maps, core_ids=list(range(NCORES)))
    _LAST_RESULTS = res
    outs = res.results

    # ---- assemble values ---------------------------------------------------
    values = np.empty(NNZ, dtype=np.float32)
    nv = []   # per-core [18, CORE_NODES] node-order summary rows
    for c in range(NCORES):
        lv = outs[c]["leafvals"]           # [16, 512, 512]
        base = c * LPC
        for l in range(LPC):
            off = st["leaf_val_off"][base + l]
            values[off:off + LEAF * LEAF] = lv[l].reshape(-1)
        nvc = np.empty((18, CORE_NODES), dtype=np.float32)
        nvc[:, st["xpos"][c]] = outs[c]["uvals"]
        nvc[:, st["mpos"][c]] = outs[c]["mmout"][:, 0:15]
        nv.append(nvc)

    core_node0 = st["core_node0"]
    for c in range(NCORES):
        for r, rec_ in enumerate(st["local_nodes"][c]):
            s = rec_["node0"] - core_node0[c]
            m = rec_["m"]
            seg = nv[c][r, s:s + m]
            values[rec_["off1"]:rec_["off1"] + m] = seg
            values[rec_["off2"]:rec_["off2"] + m] = seg
            values[rec_["one_off"]] = 1.0

    mean_row = st["mean_row"]
    for rec_ in st["upper_nodes"]:
        d = rec_["depth"]
        row = 18 - d
        c_lo = rec_["x0"] // RPC
        c_hi = (rec_["x0"] + rec_["n"]) // RPC
        for off in (rec_["off1"], rec_["off2"]):
            for c in range(c_lo, c_hi):
                dst = off + (core_node0[c] - rec_["node0"])
                values[dst:dst + CORE_NODES] = nv[c][row]
            # mean nodes of strictly deeper internal nodes within span
            for sub in st["upper_nodes"]:
                if sub["depth"] > d and rec_["x0"] <= sub["x0"] \
                        and sub["x0"] + sub["n"] <= rec_["x0"] + rec_["n"]:
                    sc, scol = mean_row[sub["mean_node"]]
                    values[off + (sub["mean_node"] - rec_["node0"])] = \
                        outs[sc]["mmout"][row, scol]
        values[rec_["one_off"]] = 1.0

    # ---- assemble all_nodes ------------------------------------------------
    nodes = np.empty((TOTAL_NODES, FMAP), dtype=np.float32)
    nodes[st["x_node_idx"]] = x2
    for c in range(NCORES):
        me = outs[c]["meansout"]           # [64, 18]
        for r, rec_ in enumerate(st["local_nodes"][c]):
            nodes[rec_["mean_node"]] = me[:, r]
    for rec_ in st["upper_nodes"]:
        sc, scol = mean_row[rec_["mean_node"]]
        nodes[rec_["mean_node"]] = outs[sc]["meansout"][:, scol]

    return nodes[None], st["indices"], values


# revision 18
# speedup vs baseline: 1.2999x; 1.0672x over previous
"""Trainium2 Bass kernel for nn_Division_Tree.

Reference semantics: a balanced binary "division tree" over X[0] of shape
[65536, 64].  128 leaves of 512 rows each get a dense 512x512 Gaussian-RBF
kernel (values, COO indices).  Every internal node appends a mean node
(mean over ALL nodes of its subtree, including previously appended means)
and emits new_v = RBF(subtree nodes, mean) twice plus a 1.0 diagonal entry.

Sharding: the 8 depth-4 subtrees (16 leaves each) map 1:1 onto the 8
NeuronCores.  Each core receives its shard pre-transposed (features major),
computes its 16 leaf kernels, its 15 local means/new_v rows, and its slice
of the depth-1..3 summary rows; only the per-core subtree sums (a [64]
vector each) are exchanged with a tiny AllGather.  COO indices are input
independent and built host-side; host assembly only places device-computed
values (no host math on X).

RBF trick used throughout: exp(-d2/64) = exp((x.y - |x|^2/2 - |y|^2/2)/32),
evaluated as two accumulated TensorE matmuls (Gram + rank-2 norm terms)
followed by a single fused ScalarE Exp(scale=1/32) straight out of PSUM.
"""

import numpy as np

# ---------------------------------------------------------------------------
# Static problem geometry (hardcoded; must match reference.py)
# ---------------------------------------------------------------------------
N = 65536
FMAP = 64
LEAF = 512            # rows per leaf (depth 8)
NLEAF = 128
NCORES = 8
LPC = NLEAF // NCORES         # 16 leaves per core
RPC = N // NCORES             # 8192 rows per core
CORE_NODES = RPC + LPC - 1  # 8207 nodes per core subtree (incl. its mean)
NNZ = 34473347
TOTAL_NODES = 65663


def _build_static():
    """Mirror of reference._dfs structure (no X dependence).

    Returns a dict of layout tables used for host-side assembly plus the
    exact COO indices array.
    """
    # ---- global recursion over the tree -----------------------------------
    leaf_val_off = []          # per leaf (in order): offset of its 512*512 block
    internal = []              # records for every internal node
    x_node_idx = np.empty(N, dtype=np.int64)   # global node index of each X row
    idx_parts = []             # pieces of the COO indices array

    def rec(x0, depth, n, node0, val0):
        """Subtree over X rows [x0, x0+n) at `depth`.

        node0/val0: global node-index / value-offset where this subtree's
        output starts.  Returns (node_count, nnz) of the subtree.
        """
        if n <= 30 or depth == 8:
            leaf_val_off.append(val0)
            x_node_idx[x0:x0 + n] = node0 + np.arange(n)
            r = np.arange(n)
            # ancestors' "+ left-node-count" shifts telescope to node0
            idx_parts.append(np.stack([np.repeat(r, n), np.tile(r, n)]) + node0)
            return n, n * n
        nl, zl = rec(x0, depth + 1, n // 2, node0, val0)
        nr, zr = rec(x0 + n // 2, depth + 1, n // 2, node0 + nl, val0 + zl)
        m = nl + nr
        ar = np.arange(m)
        full = np.full(m, m)
        idx_parts.append(np.concatenate(
            [np.stack([ar, full]), np.stack([full, ar]),
             np.array([[m], [m]])], 1) + node0)
        internal.append(dict(
            depth=depth, x0=x0, n=n, node0=node0, m=m,
            off1=val0 + zl + zr, off2=val0 + zl + zr + m,
            one_off=val0 + zl + zr + 2 * m,
            mean_node=node0 + m,
        ))
        return m + 1, zl + zr + 2 * m + 1

    total_nodes, nnz = rec(0, 1, N, 0, 0)
    assert total_nodes == TOTAL_NODES and nnz == NNZ, (total_nodes, nnz)

    # reference concatenates leaf/internal index blocks depth-first; the
    # recursion above appends idx_parts in exactly that order, but index
    # blocks of a PARENT must add the left-subtree offset to right-child
    # parts only (done above).  Base indices are subtree-local like the
    # reference (leaf blocks use local 0..n-1; parent adds offsets lazily
    # through the recursion).  The reference applies offsets the same way.
    indices = np.concatenate(idx_parts, axis=1).astype(np.int32)
    assert indices.shape == (2, NNZ)

    # ---- per-core tables ---------------------------------------------------
    # local means of core c in node-order appearance == sorted by mean_node
    local_nodes = [[] for _ in range(NCORES)]
    upper_nodes = []
    for rec_ in internal:
        if rec_["depth"] >= 4:
            c = rec_["x0"] // RPC
            local_nodes[c].append(rec_)
        else:
            upper_nodes.append(rec_)
    for c in range(NCORES):
        local_nodes[c].sort(key=lambda r: r["mean_node"])
        assert len(local_nodes[c]) == LPC - 1
    # start of each core's depth-4 subtree in global node order (upper mean
    # nodes are interleaved between core blocks, so this is NOT c*CORE_NODES)
    core_node0 = [local_nodes[c][-1]["node0"] for c in range(NCORES)]
    for c in range(NCORES):
        assert local_nodes[c][-1]["depth"] == 4
        assert local_nodes[c][-1]["mean_node"] == core_node0[c] + CORE_NODES - 1
    # mean id -> (owning first core, uvals/mm row index)
    mean_row = {}
    for c in range(NCORES):
        for r, rec_ in enumerate(local_nodes[c]):
            mean_row[rec_["mean_node"]] = (c, r)
    for rec_ in upper_nodes:
        c0 = rec_["x0"] // RPC
        mean_row[rec_["mean_node"]] = (c0, 18 - rec_["depth"])

    # node-order position of each x row / local mean inside its core block
    xpos = [x_node_idx[c * RPC:(c + 1) * RPC] - core_node0[c]
            for c in range(NCORES)]
    mpos = [np.array([r_["mean_node"] - core_node0[c]
                      for r_ in local_nodes[c]], dtype=np.int64)
            for c in range(NCORES)]

    # ---- C16: local mean/sum coefficients over the 16 leaf sums -----------
    means_coef = []

    def lrec(lo, hi, depth):
        if depth == 8:
            e = np.zeros(LPC)
            e[lo] = 1.0
            return e, 512
        mid = (lo + hi) // 2
        tl, cl = lrec(lo, mid, depth + 1)
        tr, cr = lrec(mid, hi, depth + 1)
        mean = (tl + tr) / (cl + cr)
        means_coef.append(mean)
        return tl + tr + mean, cl + cr + 1

    t4, c4 = lrec(0, LPC, 4)
    assert c4 == CORE_NODES and len(means_coef) == 15
    c16 = np.stack(means_coef + [t4], axis=1).astype(np.float32)  # [16,16]

    # ---- CU: upper mean coefficients over the 8 core sums -----------------
    upper_coef = {}   # (depth, first_core) -> coef [8]

    def urec(lo, hi, depth):
        if depth == 4:
            e = np.zeros(NCORES)
            e[lo] = 1.0
            return e, CORE_NODES
        mid = (lo + hi) // 2
        tl, cl = urec(lo, mid, depth + 1)
        tr, cr = urec(mid, hi, depth + 1)
        mean = (tl + tr) / (cl + cr)
        upper_coef[(depth, lo)] = mean
        return tl + tr + mean, cl + cr + 1

    _, c1 = urec(0, NCORES, 1)
    assert c1 == TOTAL_NODES
    cu = np.zeros((NCORES, 8, 3), dtype=np.float32)
    for c in range(NCORES):
        cu[c, :, 0] = upper_coef[(3, (c // 2) * 2)]
        cu[c, :, 1] = upper_coef[(2, (c // 4) * 4)]
        cu[c, :, 2] = upper_coef[(1, 0)]

    return dict(
        indices=indices,
        leaf_val_off=np.array(leaf_val_off, dtype=np.int64),
        local_nodes=local_nodes,
        upper_nodes=upper_nodes,
        mean_row=mean_row,
        xpos=xpos,
        mpos=mpos,
        core_node0=core_node0,
        x_node_idx=x_node_idx,
        c16=c16,
        cu=cu,
    )


_STATIC = None


def _static():
    global _STATIC
    if _STATIC is None:
        _STATIC = _build_static()
    return _STATIC


# ---------------------------------------------------------------------------
# Device kernel
# ---------------------------------------------------------------------------
_NC = None
_LAST_RESULTS = None  # test.py reads .exec_time_ns when BASS_TRACE is set


def _build_nc():
    from contextlib import ExitStack
    import concourse.tile as tile
    from concourse import bacc, mybir

    F32 = mybir.dt.float32
    F32R = mybir.dt.float32r
    EXP = mybir.ActivationFunctionType.Exp

    nc = bacc.Bacc("TRN2", target_bir_lowering=False, debug=False,
                   num_devices=NCORES)

    xt_d = nc.dram_tensor("xt", [FMAP, RPC], F32R, kind="ExternalInput").ap()
    c16_d = nc.dram_tensor("c16", [LPC, 16], F32, kind="ExternalInput").ap()
    cu_d = nc.dram_tensor("cu", [8, 3], F32, kind="ExternalInput").ap()
    eye_d = nc.dram_tensor("eye64", [64, 64], F32, kind="ExternalInput").ap()
    w2_d = nc.dram_tensor("w2", [65, 2], F32R, kind="ExternalInput").ap()
    w2l_d = nc.dram_tensor("w2l", [65, 2], F32R, kind="ExternalInput").ap()
    v2_d = nc.dram_tensor("v2", [65, 2], F32, kind="ExternalInput").ap()

    # p-major layout: [leaf, p, cc*512+n] with row cc*128+p of leaf matrix
    # at [leaf, p, cc*512:+512]; host reorders.  Gives 8KB-contiguous
    # per-partition DMA descriptors instead of 2KB interleaved.
    lv_d = nc.dram_tensor("leafvals", [LPC, 128, 4 * LEAF], F32,
                          kind="ExternalOutput").ap()
    uv_d = nc.dram_tensor("uvals", [18, RPC], F32,
                          kind="ExternalOutput").ap()
    mm_d = nc.dram_tensor("mmout", [18, 18], F32, kind="ExternalOutput").ap()
    me_d = nc.dram_tensor("meansout", [FMAP, 18], F32,
                          kind="ExternalOutput").ap()

    with tile.TileContext(nc) as tc, ExitStack() as ctx:
        sb = ctx.enter_context(tc.tile_pool(name="sb", bufs=1))
        kpool = ctx.enter_context(tc.tile_pool(name="kbig", bufs=3))
        kp = ctx.enter_context(tc.tile_pool(name="kp", bufs=3, space="PSUM"))
        up = ctx.enter_context(tc.tile_pool(name="up", bufs=2, space="PSUM"))
        sp = ctx.enter_context(tc.tile_pool(name="sp", bufs=2, space="PSUM"))
        dram = ctx.enter_context(tc.tile_pool(name="dram", bufs=1,
                                              space="DRAM"))

        # ---- persistent SBUF tiles ----------------------------------------
        xl = sb.tile([66, RPC], F32R, tag="xl")     # [x ; -0.5 ; a]
        xr = sb.tile([66, RPC], F32R, tag="xr")     # [x ; a ; -0.5]
        xsq = sb.tile([65, RPC], F32R, tag="xsq")   # [x*x ; 1]
        ls = sb.tile([64, 16], F32, tag="ls")
        lst = sb.tile([16, 64], F32, tag="lst")
        eye = sb.tile([64, 64], F32, tag="eye")
        c16s = sb.tile([16, 16], F32, tag="c16s")
        cus = sb.tile([8, 3], F32, tag="cus")
        w2s = sb.tile([65, 2], F32R, tag="w2s")
        w2ls = sb.tile([65, 2], F32R, tag="w2ls")
        v2s = sb.tile([65, 2], F32, tag="v2s")
        at4 = sb.tile([8, 64], F32, tag="at4")
        augl = sb.tile([66, 18], F32R, tag="augl")  # [means ; -0.5 ; |m|^2]
        msq65 = sb.tile([65, 18], F32, tag="msq65")
        mn2r = sb.tile([1, 18], F32, tag="mn2r")
        ones_r = sb.tile([1, 32], F32, tag="ones_r")
        ones64 = sb.tile([64, 1], F32, tag="ones64")
        t4sb = sb.tile([64, 1], F32, tag="t4sb")
        mmsb = sb.tile([18, 18], F32, tag="mmsb")
        meansf = sb.tile([64, 18], F32, tag="meansf")
        ubig = sb.tile([18, RPC], F32, tag="ubig")

        for g in range(4):
            lo = g * (RPC // 4)
            nc.sync.dma_start(xl[0:64, lo:lo + RPC // 4],
                              xt_d[:, lo:lo + RPC // 4])
        nc.sync.dma_start(eye[:], eye_d[:])
        nc.sync.dma_start(c16s[:], c16_d[:])
        nc.sync.dma_start(cus[:], cu_d[:])
        nc.sync.dma_start(w2s[:], w2_d[:])
        nc.sync.dma_start(w2ls[:], w2l_d[:])
        nc.sync.dma_start(v2s[:], v2_d[:])
        nc.gpsimd.memset(xsq[64:65, :].bitcast(mybir.dt.uint32), 0x3F800000)
        nc.gpsimd.memset(ones_r[:], 1.0)
        nc.gpsimd.memset(ones64[:], 1.0)

        # Chunked so compute starts as soon as the first DMA chunk lands;
        # LS chunks go first on the DVE queue (the collective needs them).
        for g in range(4):
            lo = g * (RPC // 4)
            nc.vector.reduce_sum(ls[:, 4 * g:4 * g + 4],
                                 xl[0:64, lo:lo + RPC // 4].bitcast(F32)
                                 .rearrange("p (l n) -> p l n", l=4),
                                 axis=mybir.AxisListType.X)
            nc.vector.tensor_mul(xsq[0:64, lo:lo + RPC // 4],
                                 xl[0:64, lo:lo + RPC // 4],
                                 xl[0:64, lo:lo + RPC // 4])
            nc.vector.tensor_copy(xr[0:64, lo:lo + RPC // 4],
                                  xl[0:64, lo:lo + RPC // 4])

        # ---- local means + core total sum ---------------------------------
        lstp = sp.tile([16, 64], F32, tag="sp")
        nc.tensor.transpose(lstp[:], ls[:], eye[:])
        nc.vector.tensor_copy(lst[:], lstp[:])
        meansp = sp.tile([64, 16], F32, tag="sp")
        nc.tensor.matmul(meansp[:], lst[:], c16s[:], start=True, stop=True)
        nc.vector.tensor_copy(augl[0:64, 0:15], meansp[:, 0:15])
        nc.vector.tensor_copy(meansf[:, 0:15], meansp[:, 0:15])
        nc.vector.tensor_copy(t4sb[:], meansp[:, 15:16])

        # ---- AllGather of core subtree sums (completes under leaf stream) --
        cc_in = dram.tile([64, 1], F32)
        cc_out = dram.tile([NCORES * 64, 1], F32)
        nc.sync.dma_start(cc_in[:], t4sb[:])
        nc.gpsimd.collective_compute(
            "AllGather", mybir.AluOpType.bypass,
            replica_groups=[list(range(NCORES))],
            ins=[cc_in.opt()], outs=[cc_out.opt()])
        nc.sync.dma_start(at4[:],
                          cc_out[:].rearrange("(c f) o -> c (f o)", c=NCORES))

        # ---- leaf RBF kernels ---------------------------------------------
        for l in range(LPC):
            lo = l * LEAF
            pt2r = sp.tile([2, LEAF], F32, tag="sp")
            nc.tensor.matmul(pt2r[:], w2s[:],
                             xsq[:, lo:lo + LEAF],
                             start=True, stop=True)
            nc.vector.tensor_copy(xr[64:66, lo:lo + LEAF], pt2r[:])
            pt2l = sp.tile([2, LEAF], F32, tag="sp")
            nc.tensor.matmul(pt2l[:], w2ls[:],
                             xsq[:, lo:lo + LEAF],
                             start=True, stop=True)
            nc.vector.tensor_copy(xl[64:66, lo:lo + LEAF], pt2l[:])

            kbig = kpool.tile([128, 4 * LEAF], F32, tag="kbig")
            for c in range(4):
                kps = kp.tile([128, LEAF], F32, tag="kp")
                col0 = lo + c * 128
                nc.tensor.matmul(kps[:], xl[:, col0:col0 + 128],
                                 xr[:, lo:lo + LEAF],
                                 start=True, stop=True)
                nc.scalar.activation(kbig[:, c * LEAF:(c + 1) * LEAF],
                                     kps[:], EXP, scale=1.0 / 32.0)
            nc.sync.dma_start(lv_d[l], kbig[:])

        # ---- upper means + mean norms (collective landed long ago) --------
        mupp = sp.tile([64, 3], F32, tag="sp")
        nc.tensor.matmul(mupp[:], at4[:], cus[:], start=True, stop=True)
        nc.vector.tensor_copy(augl[0:64, 15:18], mupp[:])
        nc.vector.tensor_copy(meansf[:, 15:18], mupp[:])

        nc.vector.tensor_mul(msq65[0:64, 0:18], augl[0:64, 0:18],
                             augl[0:64, 0:18])
        nc.gpsimd.memset(msq65[64:65, 0:18], 1.0)
        mnp = sp.tile([1, 18], F32, tag="sp")
        nc.tensor.matmul(mnp[:], ones64[:], msq65[0:64, 0:18],
                         start=True, stop=True)
        nc.vector.tensor_scalar_mul(mn2r[:], mnp[:], -0.5)
        pt2m = sp.tile([2, 18], F32, tag="sp")
        nc.tensor.matmul(pt2m[:], v2s[:], msq65[:, 0:18], start=True, stop=True)
        nc.vector.tensor_copy(augl[64:66, 0:18], pt2m[:])

        # ---- per-row summary values vs the 18 ancestor means --------------
        for ch in range(16):
            lo = ch * LEAF
            ups = up.tile([18, LEAF], F32, tag="up")
            nc.tensor.matmul(ups[:], augl[:, 0:18],
                             xr[:, lo:lo + LEAF],
                             start=True, stop=True)
            nc.scalar.activation(ubig[:, lo:lo + LEAF], ups[:], EXP,
                                 scale=1.0 / 32.0)
        nc.sync.dma_start(uv_d[:], ubig[:])

        # ---- mean-vs-mean summary values ----------------------------------
        mmp = sp.tile([18, 18], F32, tag="sp")
        nc.tensor.matmul(mmp[:], augl[0:64, 0:18], augl[0:64, 0:18],
                         start=True, stop=False)
        nc.tensor.matmul(mmp[:], mn2r[:], ones_r[0:1, 0:18],
                         start=False, stop=False)
        nc.tensor.matmul(mmp[:], ones_r[0:1, 0:18], mn2r[:],
                         start=False, stop=True)
        nc.scalar.activation(mmsb[:], mmp[:], EXP, scale=1.0 / 32.0)
        nc.sync.dma_start(mm_d[:], mmsb[:])
        nc.sync.dma_start(me_d[:], meansf[:])

    nc.compile()
    return nc


def _get_nc():
    global _NC
    if _NC is None:
        _NC = _build_nc()
    return _NC


# ---------------------------------------------------------------------------
# Host entry point
# ---------------------------------------------------------------------------
def kernel(X):
    global _LAST_RESULTS
    from concourse.bass_utils import run_bass_kernel_spmd

    st = _static()
    X = np.asarray(X)
    assert X.shape == (1, N, FMAP), X.shape
    x2 = X[0].astype(np.float32, copy=False)

    eye = np.eye(64, dtype=np.float32)
    w2 = np.zeros((65, 2), dtype=np.float32)   # xr rows 64-65 = [a ; -0.5]
    w2[0:64, 0] = 1.0
    w2[64, 1] = -0.5
    w2l = np.zeros((65, 2), dtype=np.float32)  # xl rows 64-65 = [-0.5 ; a]
    w2l[64, 0] = -0.5
    w2l[0:64, 1] = 1.0
    v2 = np.zeros((65, 2), dtype=np.float32)   # augl rows 64-65 = [-0.5 ; |m|^2]
    v2[64, 0] = -0.5
    v2[0:64, 1] = 1.0
    in_maps = []
    for c in range(NCORES):
        xt = np.ascontiguousarray(x2[c * RPC:(c + 1) * RPC].T)
        in_maps.append({"xt": xt, "c16": st["c16"], "cu": st["cu"][c],
                        "eye64": eye, "w2": w2, "w2l": w2l, "v2": v2})

    nc = _get_nc()
    res = run_bass_kernel_spmd(nc, in_maps, core_ids=list(range(NCORES)))
    _LAST_RESULTS = res
    outs = res.results

    # ---- assemble values ---------------------------------------------------
    values = np.empty(NNZ, dtype=np.float32)
    nv = []   # per-core [18, CORE_NODES] node-order summary rows
    for c in range(NCORES):
        lv = outs[c]["leafvals"].reshape(LPC, 128, 4, LEAF)
        lv = lv.transpose(0, 2, 1, 3)          # -> [16, cc, p, n] row-major
        base = c * LPC
        for l in range(LPC):
            off = st["leaf_val_off"][base + l]
            values[off:off + LEAF * LEAF] = lv[l].reshape(-1)
        nvc = np.empty((18, CORE_NODES), dtype=np.float32)
        nvc[:, st["xpos"][c]] = outs[c]["uvals"]
        nvc[:, st["mpos"][c]] = outs[c]["mmout"][:, 0:15]
        nv.append(nvc)

    core_node0 = st["core_node0"]
    for c in range(NCORES):
        for r, rec_ in enumerate(st["local_nodes"][c]):
            s = rec_["node0"] - core_node0[c]
            m = rec_["m"]
            seg = nv[c][r, s:s + m]
            values[rec_["off1"]:rec_["off1"] + m] = seg
            values[rec_["off2"]:rec_["off2"] + m] = seg
            values[rec_["one_off"]] = 1.0

    mean_row = st["mean_row"]
    for rec_ in st["upper_nodes"]:
        d = rec_["depth"]
        row = 18 - d
        c_lo = rec_["x0"] // RPC
        c_hi = (rec_["x0"] + rec_["n"]) // RPC
        for off in (rec_["off1"], rec_["off2"]):
            for c in range(c_lo, c_hi):
                dst = off + (core_node0[c] - rec_["node0"])
                values[dst:dst + CORE_NODES] = nv[c][row]
            # mean nodes of strictly deeper internal nodes within span
            for sub in st["upper_nodes"]:
                if sub["depth"] > d and rec_["x0"] <= sub["x0"] \
                        and sub["x0"] + sub["n"] <= rec_["x0"] + rec_["n"]:
                    sc, scol = mean_row[sub["mean_node"]]
                    values[off + (sub["mean_node"] - rec_["node0"])] = \
                        outs[sc]["mmout"][row, scol]
        values[rec_["one_off"]] = 1.0

    # ---- assemble all_nodes ------------------------------------------------
    nodes = np.empty((TOTAL_NODES, FMAP), dtype=np.float32)
    nodes[st["x_node_idx"]] = x2
    for c in range(NCORES):
        me = outs[c]["meansout"]           # [64, 18]
        for r, rec_ in enumerate(st["local_nodes"][c]):
            nodes[rec_["mean_node"]] = me[:, r]
    for rec_ in st["upper_nodes"]:
        sc, scol = mean_row[rec_["mean_node"]]
        nodes[rec_["mean_node"]] = outs[sc]["meansout"][:, scol]

    return nodes[None], st["indices"], values
